# revision 24
# baseline (speedup 1.0000x reference)
# Causal self-attention with RoPE, sharded over 8 TRN2 NeuronCores.
#
# Sharding: head-parallel. Each core owns 2 of the 16 heads (a 128-wide
# slice of the QKV projection output dims and of Wp's input dims) and
# computes a full [B*T, C] partial of the output projection in bf16. The
# host sums the 8 partials (the "all-reduce") and adds bp.
#
# Device program (per core), pipelined over 8 512-token spans:
#   proj(g):  q/k projections x-stationary in natural [t, d] layout with a
#     zero opener (PSUM zero-region is 2KB so per-column-block start=True
#     would clobber siblings); v weight-stationary straight into [t, d] via
#     full-tile accumulation; rope on DVE/Pool from a bf16 SBUF stage; q/k
#     PE-transposed to qT/kT [d=128, BT] bf16.  v evicted to vext fp8
#     [tk, tile, head, d+1] with a ones column (PV also produces the
#     softmax denominator).
#   S(idx):   per j-tile, both heads' S^T blocks land in one [128, 1024]
#     f32 PSUM tile (separate 2KB zero regions); one Exp per j covers both
#     heads and writes fp8 into paired pt tiles [128, head, jpar, 512];
#     causal masking via gpsimd affine_select on the diagonal (odd pair
#     members also zero the stale 128 columns left of their block).
#   PV(idx):  fp8 DoubleRow matmuls (2 k-tiles per instruction, 0.5
#     cycles/row) accumulate yT+l [65, 512]; normalization: DVE
#     reciprocal of l, gpsimd partition_broadcast, one DVE multiply
#     writing yT_sb bf16.
#   out(idx): [128, 512] f32 out-proj tiles, DVE-evicted to a bf16 stage,
#     one DMA per 512 tokens.
#   Emission interleaves S j-tiles with PV/out/proj quanta so the PE
#   stream stays dense while the Activation engine drains the exps.
import math
from collections import deque
from contextlib import ExitStack

import numpy as np
import ml_dtypes

import concourse.bass as bass
import concourse.mybir as mybir
import concourse.tile as tile
from concourse import bacc
from concourse.bass_utils import run_bass_kernel_spmd
from concourse.masks import make_identity

B, T, C, H = 2, 2048, 1024, 16
D = C // H          # 64, head dim
BT = B * T          # 4096 tokens
NCORES = 8
HPC = H // NCORES   # 2 heads per core
DPC = HPC * D       # 128 projection dims per core
NT = BT // 128      # 32 token tiles
NTB = T // 128      # 16 token tiles per batch
NS = T // 512       # 4 q-spans per batch
NSPAN = B * NS      # 8 (batch, span) pairs == 8 512-token groups

F32 = mybir.dt.float32
BF16 = mybir.dt.bfloat16
FP8 = mybir.dt.float8e4


def _rope_cache_host():
    """Bit-exact replica of the reference's jax f32 rope cache, computed on
    the CPU backend (theta/cos/sin at large angles are sensitive to the
    exact f32 implementation, so this must go through jax, not numpy)."""
    import jax
    import jax.numpy as jnp

    cpu = jax.devices("cpu")[0]
    with jax.default_device(cpu):
        i = jnp.arange(D // 2, dtype=jnp.float32)
        theta = 1.0 / (10000.0 ** (-2.0 * (i - 1.0) / D))
        ang = jnp.arange(T, dtype=jnp.float32)[:, None] * theta[None, :]
        cos = np.asarray(jnp.cos(ang))
        sin = np.asarray(jnp.sin(ang))
    return cos, sin  # [T, D/2] f32


def _build_program():
    nc = bacc.Bacc("TRN2", target_bir_lowering=False, debug=False)

    xT = nc.dram_tensor("xT", [C, BT], BF16, kind="ExternalInput").ap()
    # weights pre-swizzled on host to [128, 8*128] = "p (k d)" so the DMA is
    # fully contiguous (>=512B runs, no RMW penalty)
    wq_p = nc.dram_tensor("wq_p", [128, 8 * DPC], BF16, kind="ExternalInput").ap()
    wk_p = nc.dram_tensor("wk_p", [128, 8 * DPC], BF16, kind="ExternalInput").ap()
    wv_p = nc.dram_tensor("wv_p", [128, 8 * DPC], BF16, kind="ExternalInput").ap()
    wpT = nc.dram_tensor("wpT", [DPC, C], BF16, kind="ExternalInput").ap()
    cosP = nc.dram_tensor("cosP", [128, NTB * D], BF16, kind="ExternalInput").ap()
    sinP = nc.dram_tensor("sinP", [128, NTB * D], BF16, kind="ExternalInput").ap()
    out = nc.dram_tensor("out_p", [BT, C], BF16, kind="ExternalOutput").ap()

    with tile.TileContext(nc) as tc, ExitStack() as ctx:
        consts = ctx.enter_context(tc.tile_pool(name="consts", bufs=1))
        xpool = ctx.enter_context(tc.tile_pool(name="xpool", bufs=3))
        stgp = ctx.enter_context(tc.tile_pool(name="stgp", bufs=4))
        roptmp = ctx.enter_context(tc.tile_pool(name="roptmp", bufs=2))
        qkvn = ctx.enter_context(tc.tile_pool(name="qkvn", bufs=3))
        big = ctx.enter_context(tc.tile_pool(name="big", bufs=1))
        ppool = ctx.enter_context(tc.tile_pool(name="ppool", bufs=10))
        lpool = ctx.enter_context(tc.tile_pool(name="lpool", bufs=3))
        ostage = ctx.enter_context(tc.tile_pool(name="ostage", bufs=2))

        # PSUM budget (8 banks x 2KB/partition):
        #   p1  tag "p":  2 x [128,512] f32 slots (projections, out-proj)   2 banks
        #   s_ps tag "s": 2 x [128,1024] f32 slots (fused-head S^T)         4 banks
        #   yp  tag "ytl": 2 x [65,512] f32 slots (yT+l accumulators)       2 banks
        p1 = ctx.enter_context(tc.tile_pool(name="p1", bufs=2, space="PSUM"))
        s_ps = ctx.enter_context(tc.tile_pool(name="s_ps", bufs=2, space="PSUM"))
        yp = ctx.enter_context(tc.tile_pool(name="yp", bufs=2, space="PSUM"))

        # ---- constants ----
        ident = consts.tile([128, 128], BF16)
        make_identity(nc, ident)
        # exp shift: fp8e4 tops out at 448 and the max causal score is ~6.3
        # (exp -> 542, NaN). -1.0 keeps the max at ~200 while leaving the
        # bulk of the weight distribution out of the coarse subnormal range.
        # The softmax self-normalizes (l uses the same shifted p), so a
        # constant shift cancels exactly.
        nbias = consts.tile([128, 1], F32)
        nc.vector.memset(nbias, -1.0)
        zero_row = consts.tile([1, 128], BF16)
        nc.vector.memset(zero_row, 0.0)
        ones512 = consts.tile([1, 512], BF16)
        nc.vector.memset(ones512, 1.0)


        w_sb = {}
        for name, wt in (("q", wq_p), ("k", wk_p), ("v", wv_p)):
            w = consts.tile([128, 8, DPC], BF16, name=f"w{name}_sb")
            nc.sync.dma_start(out=w, in_=wt.rearrange("p (k d) -> p k d", d=8 * DPC // 8))
            w_sb[name] = w
        cos_sb = consts.tile([128, NTB, D], BF16)
        sin_sb = consts.tile([128, NTB, D], BF16)
        nc.sync.dma_start(out=cos_sb, in_=cosP.rearrange("p (n d) -> p n d", d=D))
        nc.sync.dma_start(out=sin_sb, in_=sinP.rearrange("p (n d) -> p n d", d=D))
        wp_sb = consts.tile([128, C], BF16)

        # persistent activations
        qT_sb = big.tile([128, BT], BF16)   # rows: [h0 d0..63, h1 d0..63]
        kT_sb = big.tile([128, BT], BF16)
        # [tk, tile, head, d+1]: col 64 is ones, so the PV matmul also
        # produces the softmax denominator in row 64 of ytl. bf16: fp8 P/V
        # each alone cost ~2.3e-2 relative error (attention is peaked, so
        # quantization does not average out) vs the 2e-2 gate.
        vext_sb = big.tile([128, NT, HPC, D + 1], BF16)
        yT_sb = big.tile([128, BT], BF16)
        nc.vector.memset(vext_sb[:, :, :, D:D + 1], 1.0)

        xT_g = xT.rearrange("(k p) (g q) -> g p k q", p=128, q=512)
        x_tiles = {}

        def load_x(g, split=False):
            x_t = xpool.tile([128, 8, 512], BF16, tag="x_t", name=f"x_t_{g}")
            if split:
                nc.sync.dma_start(out=x_t[:, 0:1, :], in_=xT_g[g, :, 0:1, :])
                nc.sync.dma_start(out=x_t[:, 1:8, :], in_=xT_g[g, :, 1:8, :])
            else:
                nc.sync.dma_start(out=x_t, in_=xT_g[g])
            x_tiles[g] = x_t

        # ---- proj(g): QKV + rope + transposes as a list of (PE-ns, fn) ----
        def proj_quanta(g):
            quanta = []
            gtb = (g % NS) * 4  # first in-batch token tile of the group

            def start(state={}):
                if g + 1 < NSPAN and (g + 1) not in x_tiles:
                    load_x(g + 1)

            quanta.append((0, start))
            st = {}

            def mk_ps(name):
                def fn():
                    st[name] = p1.tile([128, 512], F32, tag="p",
                                       name=f"ps_{name}_{g}")
                    # full-tile zero opener: per-column-block start=True would
                    # mark the whole 2KB PSUM row pending-zero
                    nc.tensor.matmul(st[name], lhsT=zero_row, rhs=ones512,
                                     start=True, stop=False)
                return fn

            def mk_qk_block(name, n):
                def fn():
                    x_t = x_tiles[g]
                    for kk in range(8):
                        nc.tensor.matmul(
                            st[name][:, n * 128:(n + 1) * 128],
                            lhsT=x_t[:, kk, n * 128:(n + 1) * 128],
                            rhs=w_sb[name][:, kk, :],
                            start=False, stop=(n == 3 and kk == 7),
                        )
                return fn

            for name in ("q", "k"):
                quanta.append((220, mk_ps(name)))
                for n in range(4):
                    quanta.append((430, mk_qk_block(name, n)))

            def evict_stg(name):
                def fn():
                    stg = stgp.tile([128, 512], BF16, tag="stg",
                                    name=f"stg_{name}_{g}")
                    nc.vector.tensor_copy(out=stg, in_=st[name])
                    st[f"stg_{name}"] = stg
                return fn

            def rope(name):
                def fn():
                    stg = st[f"stg_{name}"]
                    s4 = stg.rearrange("p (n d2) -> p n d2", n=4)
                    ev, od = s4[:, :, 0:DPC:2], s4[:, :, 1:DPC:2]
                    ct = cos_sb[:, gtb:gtb + 4, :]
                    stt = sin_sb[:, gtb:gtb + 4, :]
                    t1 = roptmp.tile([128, 4, D], BF16, tag="t1")
                    t2 = roptmp.tile([128, 4, D], BF16, tag="t2")
                    nc.vector.tensor_mul(t1, ev, ct)
                    nc.vector.tensor_mul(t2, od, stt)
                    qn = qkvn.tile([128, 512], BF16, tag="qn",
                                   name=f"{name}n_{g}")
                    qn4 = qn.rearrange("p (n d2) -> p n d2", n=4)
                    nc.vector.tensor_sub(qn4[:, :, 0:DPC:2], t1, t2)
                    t3 = roptmp.tile([128, 4, D], BF16, tag="t3")
                    t4 = roptmp.tile([128, 4, D], BF16, tag="t4")
                    nc.gpsimd.tensor_mul(t3, ev, stt)
                    nc.gpsimd.tensor_mul(t4, od, ct)
                    nc.vector.tensor_add(qn4[:, :, 1:DPC:2], t3, t4)
                    st[f"qn_{name}"] = qn
                return fn

            def transp(name, n0):
                def fn():
                    if n0 == 0:
                        st[f"tp_{name}"] = p1.tile([128, 1024], BF16, tag="p",
                                                   name=f"tp_{name}_{g}")
                    tp = st[f"tp_{name}"]
                    qn = st[f"qn_{name}"]
                    for n in (n0, n0 + 1):
                        nc.tensor.transpose(
                            tp[:, n * 128:(n + 1) * 128],
                            qn[:, n * 128:(n + 1) * 128], ident,
                        )
                return fn

            def tp_out(name):
                def fn():
                    dst = qT_sb if name == "q" else kT_sb
                    nc.vector.tensor_copy(
                        out=dst[:, g * 512:(g + 1) * 512],
                        in_=st[f"tp_{name}"][:, 0:512],
                    )
                return fn

            quanta.append((0, evict_stg("q")))
            quanta.append((0, rope("q")))
            quanta.append((220, mk_ps("v")))
            for n in range(4):
                quanta.append((430, mk_qk_block("v", n)))

            def evict_v():
                psv4 = st["v"].rearrange("p (n hh d) -> p n hh d", hh=HPC, d=D)
                nc.vector.tensor_copy(
                    out=vext_sb[:, g * 4:(g + 1) * 4, :, 0:D], in_=psv4)

            quanta.append((0, evict_v))
            quanta.append((0, evict_stg("k")))
            quanta.append((0, rope("k")))
            quanta.append((110, lambda: transp("q", 0)()))
            quanta.append((110, lambda: transp("q", 2)()))
            quanta.append((0, tp_out("q")))
            quanta.append((110, lambda: transp("k", 0)()))
            quanta.append((110, lambda: transp("k", 2)()))
            quanta.append((0, tp_out("k")))
            return quanta

        # ---- S(idx): S^T + exp + causal masks, pulling filler between js --
        def emit_span_S(idx, filler, pts_out):
            b, s = divmod(idx, NS)
            nj = 4 * s + 4
            for j in range(nj):
                dj = j - 4 * s
                coff = max(dj, 0) * 128
                n0 = 512 - coff
                if j % 2 == 0:
                    pt = ppool.tile([128, HPC, 2, 512], BF16, tag="pt",
                                    name=f"pt_{idx}_{j // 2}")
                    pts_out.append(pt)
                else:
                    pt = pts_out[-1]
                sp = s_ps.tile([128, 1024], F32, tag="s", name=f"sp_{idx}_{j}")
                for h in range(HPC):
                    rows = slice(h * D, (h + 1) * D)
                    nc.tensor.matmul(
                        sp[:, h * 512 + coff:(h + 1) * 512],
                        lhsT=kT_sb[rows, b * T + j * 128:b * T + (j + 1) * 128],
                        rhs=qT_sb[rows, b * T + s * 512 + coff:b * T + (s + 1) * 512],
                        start=True, stop=True,
                    )
                sp2 = sp.rearrange("p (h q) -> p h q", h=HPC)
                nc.scalar.activation(
                    out=pt[:, :, j % 2, coff:512], in_=sp2[:, :, coff:512],
                    func=mybir.ActivationFunctionType.Exp, bias=nbias,
                )
                if dj >= 0:
                    # causal zeroing: upper triangle of the diagonal block
                    for h in range(HPC):
                        nc.gpsimd.affine_select(
                            out=pt[:, h, j % 2, coff:coff + 128],
                            in_=pt[:, h, j % 2, coff:coff + 128],
                            compare_op=mybir.AluOpType.is_ge,
                            fill=0.0, base=0,
                            pattern=[[1, 128]], channel_multiplier=-1,
                        )
                # pull PE filler to cover the Act exp drain (~600ns/j)
                want = 600 if j < nj - 2 else 250
                got = 0
                while filler and got < want:
                    ns_est, fn = filler.popleft()
                    fn()
                    got += max(ns_est, 60)

        # ---- PV(idx) + normalization, as filler quanta ----
        def pv_quanta(idx, pts):
            b, s = divmod(idx, NS)
            nj = 4 * s + 4
            quanta = []
            st = {}

            def mk_pv(h, j):
                def fn():
                    if j == 0:
                        st[h] = yp.tile([D + 1, 512], F32, tag="ytl",
                                        name=f"ytl_{idx}_{h}")
                    coff = max(j - 4 * s, 0) * 128
                    nc.tensor.matmul(
                        st[h][:, coff:512],
                        lhsT=vext_sb[:, b * NTB + j, h, :],
                        rhs=pts[j // 2][:, h, j % 2, coff:512],
                        start=(j == 0), stop=(j == nj - 1),
                    )
                return fn

            def mk_norm(h):
                def fn():
                    ytl = st[h]
                    rcp = lpool.tile([1, 512], F32, tag="rcp",
                                     name=f"rcp_{idx}_{h}")
                    nc.vector.reciprocal(rcp, ytl[D:D + 1, :])
                    rbc = lpool.tile([D, 512], F32, tag="rbc",
                                     name=f"rbc_{idx}_{h}")
                    nc.gpsimd.partition_broadcast(rbc, rcp[0:1, :])
                    rows = slice(h * D, (h + 1) * D)
                    nc.vector.tensor_mul(
                        yT_sb[rows, b * T + s * 512:b * T + (s + 1) * 512],
                        ytl[0:D, :], rbc,
                    )
                return fn

            for h in range(HPC):
                for j in range(nj):
                    quanta.append((120, mk_pv(h, j)))
                quanta.append((0, mk_norm(h)))
            return quanta

        # ---- out(idx): output projection + eviction + DMA ----
        def out_quanta(idx):
            quanta = []
            st = {}

            def mk_half(t, e):
                def fn():
                    i = idx * 4 + t
                    if t == 0 and e == 0:
                        st["ob"] = ostage.tile([128, 4, C], BF16, tag="ob",
                                               name=f"ob_{idx}")
                    op = p1.tile([128, 512], F32, tag="p", name=f"op_{i}_{e}")
                    nc.tensor.matmul(
                        op, lhsT=yT_sb[:, i * 128:(i + 1) * 128],
                        rhs=wp_sb[:, e * 512:(e + 1) * 512],
                        start=True, stop=True,
                    )
                    nc.vector.tensor_copy(
                        out=st["ob"][:, t, e * 512:(e + 1) * 512], in_=op)
                return fn

            def dma():
                nc.sync.dma_start(
                    out=out_r[idx], in_=st["ob"])

            for t in range(4):
                for e in range(2):
                    quanta.append((215, mk_half(t, e)))
            quanta.append((0, dma))
            return quanta

        out_r = out.rearrange("(i n p) e -> i p n e", p=128, n=4)

        # ---- pipelined emission ----
        load_x(0, split=True)
        load_x(1)
        for _, fn in proj_quanta(0):
            fn()
        nc.sync.dma_start(out=wp_sb, in_=wpT)

        pts_store = {}
        for idx in range(NSPAN + 2):
            filler = deque()
            if 1 <= idx <= NSPAN:
                filler.extend(pv_quanta(idx - 1, pts_store.pop(idx - 1)))
            if 2 <= idx <= NSPAN + 1:
                filler.extend(out_quanta(idx - 2))
            if idx + 1 < NSPAN:
                filler.extend(proj_quanta(idx + 1))
            if idx < NSPAN:
                pts = []
                emit_span_S(idx, filler, pts)
                pts_store[idx] = pts
            while filler:
                _, fn = filler.popleft()
                fn()

    nc.compile()
    return nc


_nc_cache = None


def _get_program():
    global _nc_cache
    if _nc_cache is None:
        _nc_cache = _build_program()
    return _nc_cache


def _host_inputs(x, Wq, bq, Wk, bk, Wv, bv, Wp, bp):
    bf = ml_dtypes.bfloat16
    scale = 1.0 / math.sqrt(D)
    x2 = np.ascontiguousarray(np.asarray(x, np.float32).reshape(BT, C).T)  # [C, BT]
    xT_b = x2.astype(bf)
    cos, sin = _rope_cache_host()  # [T, D/2]
    cosE = np.concatenate([cos, cos], axis=1)  # [T, D] (2 heads' even cols)
    sinE = np.concatenate([sin, sin], axis=1)

    def swizzle_t(m):  # [T, D] -> [128, NTB*D] "(n p) d -> p (n d)"
        return np.ascontiguousarray(
            m.reshape(NTB, 128, D).transpose(1, 0, 2).reshape(128, NTB * D)
        ).astype(bf)

    def swizzle_w(wT):  # [C, DPC] -> [128, 8*DPC] "(k p) d -> p (k d)"
        return np.ascontiguousarray(
            wT.reshape(8, 128, DPC).transpose(1, 0, 2).reshape(128, 8 * DPC)
        ).astype(bf)

    common = {
        "xT": xT_b,
        "cosP": swizzle_t(cosE),
        "sinP": swizzle_t(sinE),
    }
    in_maps = []
    for m in range(NCORES):
        sl = slice(m * DPC, (m + 1) * DPC)
        in_maps.append({
            **common,
            "wq_p": swizzle_w((np.asarray(Wq, np.float32)[sl] * scale).T),
            "wk_p": swizzle_w(np.asarray(Wk, np.float32)[sl].T),
            "wv_p": swizzle_w(np.asarray(Wv, np.float32)[sl].T),
            "wpT": np.ascontiguousarray(
                np.asarray(Wp, np.float32)[:, sl].T).astype(bf),
        })
    return in_maps


def kernel(x, Wq, bq, Wk, bk, Wv, bv, Wp, bp, _run_kwargs=None):
    nc = _get_program()
    in_maps = _host_inputs(x, Wq, bq, Wk, bk, Wv, bv, Wp, bp)
    res = run_bass_kernel_spmd(
        nc, in_maps, core_ids=list(range(NCORES)), **(_run_kwargs or {})
    )
    partials = [r["out_p"] for r in res.results]
    acc = np.zeros((BT, C), np.float32)
    for p in partials:
        acc += np.asarray(p, dtype=np.float32)
    out = acc + np.asarray(bp, np.float32)[None, :]
    if _run_kwargs:
        kernel.last_results = res
    return out.reshape(B, T, C)


# revision 30
# speedup vs baseline: 1.0060x; 1.0060x over previous
# Causal self-attention with RoPE, sharded over 8 TRN2 NeuronCores.
#
# Sharding: head-parallel. Each core owns 2 of the 16 heads (a 128-wide
# slice of the QKV projection output dims and of Wp's input dims) and
# computes a full [B*T, C] partial of the output projection in bf16. The
# host sums the 8 partials (the "all-reduce") and adds bp.
#
# Device program (per core), pipelined over 8 512-token spans:
#   proj(g):  q/k projections x-stationary in natural [t, d] layout with a
#     zero opener (PSUM zero-region is 2KB so per-column-block start=True
#     would clobber siblings); v weight-stationary straight into [t, d] via
#     full-tile accumulation; rope on DVE/Pool from a bf16 SBUF stage; q/k
#     PE-transposed to qT/kT [d=128, BT] bf16.  v evicted to vext fp8
#     [tk, tile, head, d+1] with a ones column (PV also produces the
#     softmax denominator).
#   S(idx):   per j-tile, both heads' S^T blocks land in one [128, 1024]
#     f32 PSUM tile (separate 2KB zero regions); one Exp per j covers both
#     heads and writes fp8 into paired pt tiles [128, head, jpar, 512];
#     causal masking via gpsimd affine_select on the diagonal (odd pair
#     members also zero the stale 128 columns left of their block).
#   PV(idx):  fp8 DoubleRow matmuls (2 k-tiles per instruction, 0.5
#     cycles/row) accumulate yT+l [65, 512]; normalization: DVE
#     reciprocal of l, gpsimd partition_broadcast, one DVE multiply
#     writing yT_sb bf16.
#   out(idx): [128, 512] f32 out-proj tiles, DVE-evicted to a bf16 stage,
#     one DMA per 512 tokens.
#   Emission interleaves S j-tiles with PV/out/proj quanta so the PE
#   stream stays dense while the Activation engine drains the exps.
import math
from collections import deque
from contextlib import ExitStack

import numpy as np
import ml_dtypes

import concourse.bass as bass
import concourse.mybir as mybir
import concourse.tile as tile
from concourse import bacc
from concourse.bass_utils import run_bass_kernel_spmd
from concourse.masks import make_identity

B, T, C, H = 2, 2048, 1024, 16
D = C // H          # 64, head dim
BT = B * T          # 4096 tokens
NCORES = 8
HPC = H // NCORES   # 2 heads per core
DPC = HPC * D       # 128 projection dims per core
NT = BT // 128      # 32 token tiles
NTB = T // 128      # 16 token tiles per batch
NS = T // 512       # 4 q-spans per batch
NSPAN = B * NS      # 8 (batch, span) pairs == 8 512-token groups

F32 = mybir.dt.float32
BF16 = mybir.dt.bfloat16
FP8 = mybir.dt.float8e4


def _rope_cache_host():
    """Bit-exact replica of the reference's jax f32 rope cache, computed on
    the CPU backend (theta/cos/sin at large angles are sensitive to the
    exact f32 implementation, so this must go through jax, not numpy)."""
    import jax
    import jax.numpy as jnp

    cpu = jax.devices("cpu")[0]
    with jax.default_device(cpu):
        i = jnp.arange(D // 2, dtype=jnp.float32)
        theta = 1.0 / (10000.0 ** (-2.0 * (i - 1.0) / D))
        ang = jnp.arange(T, dtype=jnp.float32)[:, None] * theta[None, :]
        cos = np.asarray(jnp.cos(ang))
        sin = np.asarray(jnp.sin(ang))
    return cos, sin  # [T, D/2] f32


def _build_program():
    nc = bacc.Bacc("TRN2", target_bir_lowering=False, debug=False)

    xT = nc.dram_tensor("xT", [C, BT], BF16, kind="ExternalInput").ap()
    # weights pre-swizzled on host to [128, 8*128] = "p (k d)" so the DMA is
    # fully contiguous (>=512B runs, no RMW penalty)
    wq_p = nc.dram_tensor("wq_p", [128, 8 * DPC], BF16, kind="ExternalInput").ap()
    wk_p = nc.dram_tensor("wk_p", [128, 8 * DPC], BF16, kind="ExternalInput").ap()
    wv_p = nc.dram_tensor("wv_p", [128, 8 * DPC], BF16, kind="ExternalInput").ap()
    wpT = nc.dram_tensor("wpT", [DPC, C], BF16, kind="ExternalInput").ap()
    cosP = nc.dram_tensor("cosP", [128, NTB * D], BF16, kind="ExternalInput").ap()
    sinP = nc.dram_tensor("sinP", [128, NTB * D], BF16, kind="ExternalInput").ap()
    out = nc.dram_tensor("out_p", [BT, C], BF16, kind="ExternalOutput").ap()

    with tile.TileContext(nc) as tc, ExitStack() as ctx:
        consts = ctx.enter_context(tc.tile_pool(name="consts", bufs=1))
        xpool = ctx.enter_context(tc.tile_pool(name="xpool", bufs=3))
        stgp = ctx.enter_context(tc.tile_pool(name="stgp", bufs=4))
        roptmp = ctx.enter_context(tc.tile_pool(name="roptmp", bufs=2))
        qkvn = ctx.enter_context(tc.tile_pool(name="qkvn", bufs=3))
        big = ctx.enter_context(tc.tile_pool(name="big", bufs=1))
        ppool = ctx.enter_context(tc.tile_pool(name="ppool", bufs=10))
        lpool = ctx.enter_context(tc.tile_pool(name="lpool", bufs=3))
        ostage = ctx.enter_context(tc.tile_pool(name="ostage", bufs=2))

        # PSUM budget (8 banks x 2KB/partition):
        #   p1  tag "p":  2 x [128,512] f32 slots (projections, out-proj)   2 banks
        #   s_ps tag "s": 2 x [128,1024] f32 slots (fused-head S^T)         4 banks
        #   yp  tag "ytl": 2 x [65,512] f32 slots (yT+l accumulators)       2 banks
        p1 = ctx.enter_context(tc.tile_pool(name="p1", bufs=2, space="PSUM"))
        s_ps = ctx.enter_context(tc.tile_pool(name="s_ps", bufs=2, space="PSUM"))
        yp = ctx.enter_context(tc.tile_pool(name="yp", bufs=2, space="PSUM"))

        # ---- constants ----
        ident = consts.tile([128, 128], BF16)
        make_identity(nc, ident)
        # exp shift: fp8e4 tops out at 448 and the max causal score is ~6.3
        # (exp -> 542, NaN). -1.0 keeps the max at ~200 while leaving the
        # bulk of the weight distribution out of the coarse subnormal range.
        # The softmax self-normalizes (l uses the same shifted p), so a
        # constant shift cancels exactly.
        nbias = consts.tile([128, 1], F32)
        nc.vector.memset(nbias, -1.0)
        zero_row = consts.tile([1, 128], BF16)
        nc.vector.memset(zero_row, 0.0)
        ones512 = consts.tile([1, 512], BF16)
        nc.vector.memset(ones512, 1.0)


        w_sb = {}
        for name, wt in (("q", wq_p), ("k", wk_p), ("v", wv_p)):
            w = consts.tile([128, 8, DPC], BF16, name=f"w{name}_sb")
            nc.sync.dma_start(out=w, in_=wt.rearrange("p (k d) -> p k d", d=8 * DPC // 8))
            w_sb[name] = w
        cos_sb = consts.tile([128, NTB, HPC, 32], BF16)
        sin_sb = consts.tile([128, NTB, HPC, 32], BF16)
        nc.sync.dma_start(out=cos_sb,
                          in_=cosP.rearrange("p (n h d) -> p n h d", h=HPC, d=32))
        nc.sync.dma_start(out=sin_sb,
                          in_=sinP.rearrange("p (n h d) -> p n h d", h=HPC, d=32))
        wp_sb = consts.tile([128, C], BF16)

        # persistent activations
        qT_sb = big.tile([128, BT], BF16)   # rows: [h0 d0..63, h1 d0..63]
        kT_sb = big.tile([128, BT], BF16)
        # [tk, tile, head, d+1]: col 64 is ones, so the PV matmul also
        # produces the softmax denominator in row 64 of ytl. bf16: fp8 P/V
        # each alone cost ~2.3e-2 relative error (attention is peaked, so
        # quantization does not average out) vs the 2e-2 gate.
        vext_sb = big.tile([128, NT, HPC, D + 1], BF16)
        yT_sb = big.tile([128, BT], BF16)
        nc.vector.memset(vext_sb[:, :, :, D:D + 1], 1.0)

        xT_g = xT.rearrange("(k p) (g q) -> g p k q", p=128, q=512)
        x_tiles = {}

        def load_x(g, split=False):
            x_t = xpool.tile([128, 8, 512], BF16, tag="x_t", name=f"x_t_{g}")
            if split:
                nc.sync.dma_start(out=x_t[:, 0:1, :], in_=xT_g[g, :, 0:1, :])
                nc.sync.dma_start(out=x_t[:, 1:8, :], in_=xT_g[g, :, 1:8, :])
            else:
                nc.sync.dma_start(out=x_t, in_=xT_g[g])
            x_tiles[g] = x_t

        # ---- proj(g): QKV + rope + transposes as a list of (PE-ns, fn) ----
        def proj_quanta(g):
            quanta = []
            gtb = (g % NS) * 4  # first in-batch token tile of the group

            def start(state={}):
                if g + 1 < NSPAN and (g + 1) not in x_tiles:
                    load_x(g + 1)

            quanta.append((0, start))
            st = {}

            def mk_ps(name):
                def fn():
                    st[name] = p1.tile([128, 512], F32, tag="p",
                                       name=f"ps_{name}_{g}")
                    # full-tile zero opener: per-column-block start=True would
                    # mark the whole 2KB PSUM row pending-zero
                    nc.tensor.matmul(st[name], lhsT=zero_row, rhs=ones512,
                                     start=True, stop=False)
                return fn

            def mk_qk_block(name, n):
                def fn():
                    x_t = x_tiles[g]
                    for kk in range(8):
                        nc.tensor.matmul(
                            st[name][:, n * 128:(n + 1) * 128],
                            lhsT=x_t[:, kk, n * 128:(n + 1) * 128],
                            rhs=w_sb[name][:, kk, :],
                            start=False, stop=(n == 3 and kk == 7),
                        )
                return fn

            for name in ("q", "k"):
                quanta.append((220, mk_ps(name)))
                for n in range(4):
                    quanta.append((430, mk_qk_block(name, n)))

            def evict_stg(name):
                def fn():
                    # de-interleave even/odd rope pairs during eviction so
                    # every DVE rope op below runs on packed views (2x mode).
                    # Within each head the d-rows become [evens, odds] — the
                    # same permutation for q and k, so scores are invariant.
                    stg = stgp.tile([128, 4, HPC, 2, 32], BF16, tag="stg",
                                    name=f"stg_{name}_{g}")
                    ps4 = st[name].rearrange("p (n h d) -> p n h d", n=4, h=HPC)
                    nc.vector.tensor_copy(out=stg[:, :, :, 0, :],
                                          in_=ps4[:, :, :, 0:D:2])
                    nc.vector.tensor_copy(out=stg[:, :, :, 1, :],
                                          in_=ps4[:, :, :, 1:D:2])
                    st[f"stg_{name}"] = stg
                return fn

            def rope(name):
                def fn():
                    stg = st[f"stg_{name}"]
                    ev, od = stg[:, :, :, 0, :], stg[:, :, :, 1, :]
                    ct = cos_sb[:, gtb:gtb + 4, :, :]
                    stt = sin_sb[:, gtb:gtb + 4, :, :]
                    t1 = roptmp.tile([128, 4, HPC, 32], BF16, tag="t1")
                    t2 = roptmp.tile([128, 4, HPC, 32], BF16, tag="t2")
                    nc.vector.tensor_mul(t1, ev, ct)
                    nc.vector.tensor_mul(t2, od, stt)
                    qn = qkvn.tile([128, 4, HPC, 2, 32], BF16, tag="qn",
                                   name=f"{name}n_{g}")
                    nc.vector.tensor_sub(qn[:, :, :, 0, :], t1, t2)
                    t3 = roptmp.tile([128, 4, HPC, 32], BF16, tag="t3")
                    t4 = roptmp.tile([128, 4, HPC, 32], BF16, tag="t4")
                    nc.gpsimd.tensor_mul(t3, ev, stt)
                    nc.gpsimd.tensor_mul(t4, od, ct)
                    nc.vector.tensor_add(qn[:, :, :, 1, :], t3, t4)
                    st[f"qn_{name}"] = qn.rearrange("p n h e d -> p (n h e d)")
                return fn

            def transp(name, n0):
                def fn():
                    if n0 == 0:
                        st[f"tp_{name}"] = p1.tile([128, 1024], BF16, tag="p",
                                                   name=f"tp_{name}_{g}")
                    tp = st[f"tp_{name}"]
                    qn = st[f"qn_{name}"]
                    for n in (n0, n0 + 1):
                        nc.tensor.transpose(
                            tp[:, n * 128:(n + 1) * 128],
                            qn[:, n * 128:(n + 1) * 128], ident,
                        )
                return fn

            def tp_out(name):
                def fn():
                    dst = qT_sb if name == "q" else kT_sb
                    nc.vector.tensor_copy(
                        out=dst[:, g * 512:(g + 1) * 512],
                        in_=st[f"tp_{name}"][:, 0:512],
                    )
                return fn

            quanta.append((0, evict_stg("q")))
            quanta.append((0, rope("q")))
            quanta.append((220, mk_ps("v")))
            for n in range(4):
                quanta.append((430, mk_qk_block("v", n)))

            def evict_v():
                psv4 = st["v"].rearrange("p (n hh d) -> p n hh d", hh=HPC, d=D)
                nc.vector.tensor_copy(
                    out=vext_sb[:, g * 4:(g + 1) * 4, :, 0:D], in_=psv4)

            quanta.append((0, evict_v))
            quanta.append((0, evict_stg("k")))
            quanta.append((0, rope("k")))
            quanta.append((110, lambda: transp("q", 0)()))
            quanta.append((110, lambda: transp("q", 2)()))
            quanta.append((0, tp_out("q")))
            quanta.append((110, lambda: transp("k", 0)()))
            quanta.append((110, lambda: transp("k", 2)()))
            quanta.append((0, tp_out("k")))
            return quanta

        # ---- S(idx): S^T + exp + causal masks, pulling filler between js --
        def emit_span_S(idx, filler, pts_out):
            b, s = divmod(idx, NS)
            nj = 4 * s + 4
            for j in range(nj):
                dj = j - 4 * s
                coff = max(dj, 0) * 128
                n0 = 512 - coff
                if j % 2 == 0:
                    pt = ppool.tile([128, HPC, 2, 512], BF16, tag="pt",
                                    name=f"pt_{idx}_{j // 2}")
                    pts_out.append(pt)
                else:
                    pt = pts_out[-1]
                sp = s_ps.tile([128, 1024], F32, tag="s", name=f"sp_{idx}_{j}")
                for h in range(HPC):
                    rows = slice(h * D, (h + 1) * D)
                    nc.tensor.matmul(
                        sp[:, h * 512 + coff:(h + 1) * 512],
                        lhsT=kT_sb[rows, b * T + j * 128:b * T + (j + 1) * 128],
                        rhs=qT_sb[rows, b * T + s * 512 + coff:b * T + (s + 1) * 512],
                        start=True, stop=True,
                    )
                sp2 = sp.rearrange("p (h q) -> p h q", h=HPC)
                nc.scalar.activation(
                    out=pt[:, :, j % 2, coff:512], in_=sp2[:, :, coff:512],
                    func=mybir.ActivationFunctionType.Exp, bias=nbias,
                )
                if dj >= 0:
                    # causal zeroing: upper triangle of the diagonal block
                    for h in range(HPC):
                        nc.gpsimd.affine_select(
                            out=pt[:, h, j % 2, coff:coff + 128],
                            in_=pt[:, h, j % 2, coff:coff + 128],
                            compare_op=mybir.AluOpType.is_ge,
                            fill=0.0, base=0,
                            pattern=[[1, 128]], channel_multiplier=-1,
                        )
                # pull PE filler to cover the Act exp drain (~600ns/j)
                want = 600 if j < nj - 2 else 250
                got = 0
                while filler and got < want:
                    ns_est, fn = filler.popleft()
                    fn()
                    got += max(ns_est, 60)

        # ---- PV(idx) + normalization, as filler quanta ----
        def pv_quanta(idx, pts):
            b, s = divmod(idx, NS)
            nj = 4 * s + 4
            quanta = []
            st = {}

            def mk_pv(h, j):
                def fn():
                    if j == 0:
                        st[h] = yp.tile([D + 1, 512], F32, tag="ytl",
                                        name=f"ytl_{idx}_{h}")
                    coff = max(j - 4 * s, 0) * 128
                    nc.tensor.matmul(
                        st[h][:, coff:512],
                        lhsT=vext_sb[:, b * NTB + j, h, :],
                        rhs=pts[j // 2][:, h, j % 2, coff:512],
                        start=(j == 0), stop=(j == nj - 1),
                    )
                return fn

            def mk_norm(h):
                def fn():
                    ytl = st[h]
                    rcp = lpool.tile([1, 512], F32, tag="rcp",
                                     name=f"rcp_{idx}_{h}")
                    nc.vector.reciprocal(rcp, ytl[D:D + 1, :])
                    rbc = lpool.tile([D, 512], F32, tag="rbc",
                                     name=f"rbc_{idx}_{h}")
                    nc.gpsimd.partition_broadcast(rbc, rcp[0:1, :])
                    rows = slice(h * D, (h + 1) * D)
                    nc.vector.tensor_mul(
                        yT_sb[rows, b * T + s * 512:b * T + (s + 1) * 512],
                        ytl[0:D, :], rbc,
                    )
                return fn

            for h in range(HPC):
                for j in range(nj):
                    quanta.append((120, mk_pv(h, j)))
                quanta.append((0, mk_norm(h)))
            return quanta

        # ---- out(idx): output projection + eviction + DMA ----
        def out_quanta(idx):
            quanta = []
            st = {}

            def mk_half(t, e):
                def fn():
                    i = idx * 4 + t
                    if t == 0 and e == 0:
                        st["ob"] = ostage.tile([128, 4, C], BF16, tag="ob",
                                               name=f"ob_{idx}")
                    op = p1.tile([128, 512], F32, tag="p", name=f"op_{i}_{e}")
                    nc.tensor.matmul(
                        op, lhsT=yT_sb[:, i * 128:(i + 1) * 128],
                        rhs=wp_sb[:, e * 512:(e + 1) * 512],
                        start=True, stop=True,
                    )
                    dst = st["ob"][:, t, e * 512:(e + 1) * 512]
                    if (2 * t + e) % 2 == 0:
                        nc.scalar.copy(out=dst, in_=op)
                    else:
                        nc.vector.tensor_copy(out=dst, in_=op)
                return fn

            def dma():
                nc.sync.dma_start(
                    out=out_r[idx], in_=st["ob"])

            for t in range(4):
                for e in range(2):
                    quanta.append((215, mk_half(t, e)))
            quanta.append((0, dma))
            return quanta

        out_r = out.rearrange("(i n p) e -> i p n e", p=128, n=4)

        # ---- pipelined emission ----
        load_x(0, split=True)
        load_x(1)
        for _, fn in proj_quanta(0):
            fn()
        nc.sync.dma_start(out=wp_sb, in_=wpT)

        pts_store = {}
        for idx in range(NSPAN + 2):
            filler = deque()
            if 1 <= idx <= NSPAN:
                filler.extend(pv_quanta(idx - 1, pts_store.pop(idx - 1)))
            if 2 <= idx <= NSPAN + 1:
                filler.extend(out_quanta(idx - 2))
            if idx + 1 < NSPAN:
                filler.extend(proj_quanta(idx + 1))
            if idx < NSPAN:
                pts = []
                emit_span_S(idx, filler, pts)
                pts_store[idx] = pts
            while filler:
                _, fn = filler.popleft()
                fn()

    nc.compile()
    return nc


_nc_cache = None


def _get_program():
    global _nc_cache
    if _nc_cache is None:
        _nc_cache = _build_program()
    return _nc_cache


def _host_inputs(x, Wq, bq, Wk, bk, Wv, bv, Wp, bp):
    bf = ml_dtypes.bfloat16
    scale = 1.0 / math.sqrt(D)
    x2 = np.ascontiguousarray(np.asarray(x, np.float32).reshape(BT, C).T)  # [C, BT]
    xT_b = x2.astype(bf)
    cos, sin = _rope_cache_host()  # [T, D/2]

    def swizzle_t(m):  # [T, 32] -> [128, NTB*HPC*32] "(n p) d -> p (n h d)"
        t = m.reshape(NTB, 128, 1, 32).transpose(1, 0, 2, 3)
        t = np.broadcast_to(t, (128, NTB, HPC, 32))
        return np.ascontiguousarray(t.reshape(128, NTB * HPC * 32)).astype(bf)

    def swizzle_w(wT):  # [C, DPC] -> [128, 8*DPC] "(k p) d -> p (k d)"
        return np.ascontiguousarray(
            wT.reshape(8, 128, DPC).transpose(1, 0, 2).reshape(128, 8 * DPC)
        ).astype(bf)

    common = {
        "xT": xT_b,
        "cosP": swizzle_t(cos),
        "sinP": swizzle_t(sin),
    }
    in_maps = []
    for m in range(NCORES):
        sl = slice(m * DPC, (m + 1) * DPC)
        in_maps.append({
            **common,
            "wq_p": swizzle_w((np.asarray(Wq, np.float32)[sl] * scale).T),
            "wk_p": swizzle_w(np.asarray(Wk, np.float32)[sl].T),
            "wv_p": swizzle_w(np.asarray(Wv, np.float32)[sl].T),
            "wpT": np.ascontiguousarray(
                np.asarray(Wp, np.float32)[:, sl].T).astype(bf),
        })
    return in_maps


def kernel(x, Wq, bq, Wk, bk, Wv, bv, Wp, bp, _run_kwargs=None):
    nc = _get_program()
    in_maps = _host_inputs(x, Wq, bq, Wk, bk, Wv, bv, Wp, bp)
    res = run_bass_kernel_spmd(
        nc, in_maps, core_ids=list(range(NCORES)), **(_run_kwargs or {})
    )
    partials = [r["out_p"] for r in res.results]
    acc = np.zeros((BT, C), np.float32)
    for p in partials:
        acc += np.asarray(p, dtype=np.float32)
    out = acc + np.asarray(bp, np.float32)[None, :]
    if _run_kwargs:
        kernel.last_results = res
    return out.reshape(B, T, C)


# revision 36
# speedup vs baseline: 1.0312x; 1.0250x over previous
# Causal self-attention with RoPE, sharded over 8 TRN2 NeuronCores.
#
# Sharding: head-parallel. Each core owns 2 of the 16 heads (a 128-wide
# slice of the QKV projection output dims and of Wp's input dims) and
# computes a full [B*T, C] partial of the output projection in bf16. The
# host sums the 8 partials (the "all-reduce") and adds bp.
#
# Device program (per core), pipelined over 8 512-token spans:
#   proj(g):  q/k projections x-stationary in natural [t, d] layout with a
#     zero opener (PSUM zero-region is 2KB so per-column-block start=True
#     would clobber siblings); v weight-stationary straight into [t, d] via
#     full-tile accumulation; rope on DVE/Pool from a bf16 SBUF stage; q/k
#     PE-transposed to qT/kT [d=128, BT] bf16.  v evicted to vext fp8
#     [tk, tile, head, d+1] with a ones column (PV also produces the
#     softmax denominator).
#   S(idx):   per j-tile, both heads' S^T blocks land in one [128, 1024]
#     f32 PSUM tile (separate 2KB zero regions); one Exp per j covers both
#     heads and writes fp8 into paired pt tiles [128, head, jpar, 512];
#     causal masking via gpsimd affine_select on the diagonal (odd pair
#     members also zero the stale 128 columns left of their block).
#   PV(idx):  fp8 DoubleRow matmuls (2 k-tiles per instruction, 0.5
#     cycles/row) accumulate yT+l [65, 512]; normalization: DVE
#     reciprocal of l, gpsimd partition_broadcast, one DVE multiply
#     writing yT_sb bf16.
#   out(idx): [128, 512] f32 out-proj tiles, DVE-evicted to a bf16 stage,
#     one DMA per 512 tokens.
#   Emission interleaves S j-tiles with PV/out/proj quanta so the PE
#   stream stays dense while the Activation engine drains the exps.
import math
from collections import deque
from contextlib import ExitStack

import numpy as np
import ml_dtypes

import concourse.bass as bass
import concourse.mybir as mybir
import concourse.tile as tile
from concourse import bacc
from concourse.bass_utils import run_bass_kernel_spmd
from concourse.masks import make_identity

B, T, C, H = 2, 2048, 1024, 16
D = C // H          # 64, head dim
BT = B * T          # 4096 tokens
NCORES = 8
HPC = H // NCORES   # 2 heads per core
DPC = HPC * D       # 128 projection dims per core
NT = BT // 128      # 32 token tiles
NTB = T // 128      # 16 token tiles per batch
NS = T // 512       # 4 q-spans per batch
NSPAN = B * NS      # 8 (batch, span) pairs == 8 512-token groups

F32 = mybir.dt.float32
BF16 = mybir.dt.bfloat16
FP8 = mybir.dt.float8e4


def _rope_cache_host():
    """Bit-exact replica of the reference's jax f32 rope cache, computed on
    the CPU backend (theta/cos/sin at large angles are sensitive to the
    exact f32 implementation, so this must go through jax, not numpy)."""
    import jax
    import jax.numpy as jnp

    cpu = jax.devices("cpu")[0]
    with jax.default_device(cpu):
        i = jnp.arange(D // 2, dtype=jnp.float32)
        theta = 1.0 / (10000.0 ** (-2.0 * (i - 1.0) / D))
        ang = jnp.arange(T, dtype=jnp.float32)[:, None] * theta[None, :]
        cos = np.asarray(jnp.cos(ang))
        sin = np.asarray(jnp.sin(ang))
    return cos, sin  # [T, D/2] f32


def _build_program():
    nc = bacc.Bacc("TRN2", target_bir_lowering=False, debug=False)

    xT = nc.dram_tensor("xT", [C, BT], BF16, kind="ExternalInput").ap()
    # weights pre-swizzled on host to [128, 8*128] = "p (k d)" so the DMA is
    # fully contiguous (>=512B runs, no RMW penalty)
    wq_p = nc.dram_tensor("wq_p", [128, 8 * DPC], BF16, kind="ExternalInput").ap()
    wk_p = nc.dram_tensor("wk_p", [128, 8 * DPC], BF16, kind="ExternalInput").ap()
    wv_p = nc.dram_tensor("wv_p", [128, 8 * DPC], BF16, kind="ExternalInput").ap()
    wpT = nc.dram_tensor("wpT", [DPC, C], BF16, kind="ExternalInput").ap()
    cosP = nc.dram_tensor("cosP", [128, NTB * D], BF16, kind="ExternalInput").ap()
    sinP = nc.dram_tensor("sinP", [128, NTB * D], BF16, kind="ExternalInput").ap()
    out = nc.dram_tensor("out_p", [BT, C], BF16, kind="ExternalOutput").ap()

    with tile.TileContext(nc) as tc, ExitStack() as ctx:
        consts = ctx.enter_context(tc.tile_pool(name="consts", bufs=1))
        xpool = ctx.enter_context(tc.tile_pool(name="xpool", bufs=3))
        stgp = ctx.enter_context(tc.tile_pool(name="stgp", bufs=4))
        roptmp = ctx.enter_context(tc.tile_pool(name="roptmp", bufs=2))
        qkvn = ctx.enter_context(tc.tile_pool(name="qkvn", bufs=3))
        big = ctx.enter_context(tc.tile_pool(name="big", bufs=1))
        ppool = ctx.enter_context(tc.tile_pool(name="ppool", bufs=10))
        lpool = ctx.enter_context(tc.tile_pool(name="lpool", bufs=3))
        ostage = ctx.enter_context(tc.tile_pool(name="ostage", bufs=2))

        # PSUM budget (8 banks x 2KB/partition):
        #   p1  tag "p":  2 x [128,512] f32 slots (projections, out-proj)   2 banks
        #   s_ps tag "s": 2 x [128,1024] f32 slots (fused-head S^T)         4 banks
        #   yp  tag "ytl": 2 x [65,512] f32 slots (yT+l accumulators)       2 banks
        p1 = ctx.enter_context(tc.tile_pool(name="p1", bufs=2, space="PSUM"))
        s_ps = ctx.enter_context(tc.tile_pool(name="s_ps", bufs=2, space="PSUM"))
        yp = ctx.enter_context(tc.tile_pool(name="yp", bufs=2, space="PSUM"))

        # ---- constants ----
        ident = consts.tile([128, 128], BF16)
        make_identity(nc, ident)
        # exp shift: fp8e4 tops out at 448 and the max causal score is ~6.3
        # (exp -> 542, NaN). -1.0 keeps the max at ~200 while leaving the
        # bulk of the weight distribution out of the coarse subnormal range.
        # The softmax self-normalizes (l uses the same shifted p), so a
        # constant shift cancels exactly.
        nbias = consts.tile([128, 1], F32)
        nc.vector.memset(nbias, -1.0)
        zero_row = consts.tile([1, 128], BF16)
        nc.vector.memset(zero_row, 0.0)
        ones512 = consts.tile([1, 512], BF16)
        nc.vector.memset(ones512, 1.0)


        w_sb = {}
        for name, wt in (("q", wq_p), ("k", wk_p), ("v", wv_p)):
            w_sb[name] = consts.tile([128, 8, DPC], BF16, name=f"w{name}_sb")
        w_srcs = {"q": wq_p, "k": wk_p, "v": wv_p}
        cos_sb = consts.tile([128, NTB, HPC, 32], BF16)
        sin_sb = consts.tile([128, NTB, HPC, 32], BF16)
        wp_sb = consts.tile([128, C], BF16)

        def emit_const_dmas(seq):
            # interleaved with the first x chunks so proj(0) can start early
            for name in seq:
                if name in w_srcs:
                    nc.sync.dma_start(
                        out=w_sb[name],
                        in_=w_srcs[name].rearrange("p (k d) -> p k d", d=DPC))
                elif name == "cos":
                    nc.sync.dma_start(
                        out=cos_sb,
                        in_=cosP.rearrange("p (n h d) -> p n h d", h=HPC, d=32))
                elif name == "sin":
                    nc.sync.dma_start(
                        out=sin_sb,
                        in_=sinP.rearrange("p (n h d) -> p n h d", h=HPC, d=32))

        # persistent activations
        qT_sb = big.tile([128, BT], BF16)   # rows: [h0 d0..63, h1 d0..63]
        kT_sb = big.tile([128, BT], BF16)
        # [tk, tile, head, d+1]: col 64 is ones, so the PV matmul also
        # produces the softmax denominator in row 64 of ytl. bf16: fp8 P/V
        # each alone cost ~2.3e-2 relative error (attention is peaked, so
        # quantization does not average out) vs the 2e-2 gate.
        vext_sb = big.tile([128, NT, HPC, D + 1], BF16)
        yT_sb = big.tile([128, BT], BF16)
        nc.vector.memset(vext_sb[:, :, :, D:D + 1], 1.0)

        xT_g = xT.rearrange("(k p) (g q) -> g p k q", p=128, q=512)
        x_tiles = {}

        def load_x(g, chunks=((0, 8),)):
            x_t = xpool.tile([128, 8, 512], BF16, tag="x_t", name=f"x_t_{g}")
            for k0, k1 in chunks:
                nc.sync.dma_start(out=x_t[:, k0:k1, :], in_=xT_g[g, :, k0:k1, :])
            x_tiles[g] = x_t

        # ---- proj(g): QKV + rope + transposes as a list of (PE-ns, fn) ----
        def proj_quanta(g):
            quanta = []
            gtb = (g % NS) * 4  # first in-batch token tile of the group

            def start(state={}):
                if g + 1 < NSPAN and (g + 1) not in x_tiles:
                    load_x(g + 1)

            quanta.append((0, start))
            st = {}

            def mk_ps(name):
                def fn():
                    st[name] = p1.tile([128, 512], F32, tag="p",
                                       name=f"ps_{name}_{g}")
                    # full-tile zero opener: per-column-block start=True would
                    # mark the whole 2KB PSUM row pending-zero
                    nc.tensor.matmul(st[name], lhsT=zero_row, rhs=ones512,
                                     start=True, stop=False)
                return fn

            def mk_qk_block(name, n):
                def fn():
                    x_t = x_tiles[g]
                    for kk in range(8):
                        nc.tensor.matmul(
                            st[name][:, n * 128:(n + 1) * 128],
                            lhsT=x_t[:, kk, n * 128:(n + 1) * 128],
                            rhs=w_sb[name][:, kk, :],
                            start=False, stop=(n == 3 and kk == 7),
                        )
                return fn

            def evict_stg(name):
                def fn():
                    # de-interleave even/odd rope pairs during eviction so
                    # every DVE rope op below runs on packed views (2x mode).
                    # Within each head the d-rows become [evens, odds] — the
                    # same permutation for q and k, so scores are invariant.
                    stg = stgp.tile([128, 4, HPC, 2, 32], BF16, tag="stg",
                                    name=f"stg_{name}_{g}")
                    ps4 = st[name].rearrange("p (n h d) -> p n h d", n=4, h=HPC)
                    nc.vector.tensor_copy(out=stg[:, :, :, 0, :],
                                          in_=ps4[:, :, :, 0:D:2])
                    nc.vector.tensor_copy(out=stg[:, :, :, 1, :],
                                          in_=ps4[:, :, :, 1:D:2])
                    st[f"stg_{name}"] = stg
                return fn

            def rope(name):
                def fn():
                    stg = st[f"stg_{name}"]
                    ev, od = stg[:, :, :, 0, :], stg[:, :, :, 1, :]
                    ct = cos_sb[:, gtb:gtb + 4, :, :]
                    stt = sin_sb[:, gtb:gtb + 4, :, :]
                    t1 = roptmp.tile([128, 4, HPC, 32], BF16, tag="t1")
                    t2 = roptmp.tile([128, 4, HPC, 32], BF16, tag="t2")
                    nc.vector.tensor_mul(t1, ev, ct)
                    nc.vector.tensor_mul(t2, od, stt)
                    qn = qkvn.tile([128, 4, HPC, 2, 32], BF16, tag="qn",
                                   name=f"{name}n_{g}")
                    nc.vector.tensor_sub(qn[:, :, :, 0, :], t1, t2)
                    t3 = roptmp.tile([128, 4, HPC, 32], BF16, tag="t3")
                    t4 = roptmp.tile([128, 4, HPC, 32], BF16, tag="t4")
                    nc.gpsimd.tensor_mul(t3, ev, stt)
                    nc.gpsimd.tensor_mul(t4, od, ct)
                    nc.vector.tensor_add(qn[:, :, :, 1, :], t3, t4)
                    st[f"qn_{name}"] = qn.rearrange("p n h e d -> p (n h e d)")
                return fn

            def transp(name, n0):
                def fn():
                    if n0 == 0:
                        st[f"tp_{name}"] = p1.tile([128, 1024], BF16, tag="p",
                                                   name=f"tp_{name}_{g}")
                    tp = st[f"tp_{name}"]
                    qn = st[f"qn_{name}"]
                    for n in (n0, n0 + 1):
                        nc.tensor.transpose(
                            tp[:, n * 128:(n + 1) * 128],
                            qn[:, n * 128:(n + 1) * 128], ident,
                        )
                return fn

            def tp_out(name):
                def fn():
                    dst = qT_sb if name == "q" else kT_sb
                    nc.vector.tensor_copy(
                        out=dst[:, g * 512:(g + 1) * 512],
                        in_=st[f"tp_{name}"][:, 0:512],
                    )
                return fn

            def evict_v():
                psv4 = st["v"].rearrange("p (n hh d) -> p n hh d", hh=HPC, d=D)
                nc.vector.tensor_copy(
                    out=vext_sb[:, g * 4:(g + 1) * 4, :, 0:D], in_=psv4)

            # order: each tensor's eviction+rope immediately after its own
            # matmuls, transposes last — maximizes the emission distance
            # between a rope (DVE/Pool chain, ~2.5us latency) and the PE
            # transpose that consumes it.
            for name in ("q", "k"):
                quanta.append((220, mk_ps(name)))
                for n in range(4):
                    quanta.append((430, mk_qk_block(name, n)))
                quanta.append((0, evict_stg(name)))
                quanta.append((0, rope(name)))
            quanta.append((220, mk_ps("v")))
            for n in range(4):
                quanta.append((430, mk_qk_block("v", n)))
            quanta.append((0, evict_v))
            quanta.append((110, transp("q", 0)))
            quanta.append((110, transp("q", 2)))
            quanta.append((0, tp_out("q")))
            quanta.append((110, transp("k", 0)))
            quanta.append((110, transp("k", 2)))
            quanta.append((0, tp_out("k")))
            return quanta

        # ---- S(idx): S^T + exp + causal masks, pulling filler between js --
        def emit_span_S(idx, filler, pts_out):
            b, s = divmod(idx, NS)
            nj = 4 * s + 4
            for j in range(nj):
                dj = j - 4 * s
                coff = max(dj, 0) * 128
                n0 = 512 - coff
                if j % 2 == 0:
                    pt = ppool.tile([128, HPC, 2, 512], BF16, tag="pt",
                                    name=f"pt_{idx}_{j // 2}")
                    pts_out.append(pt)
                else:
                    pt = pts_out[-1]
                sp = s_ps.tile([128, 1024], F32, tag="s", name=f"sp_{idx}_{j}")
                for h in range(HPC):
                    rows = slice(h * D, (h + 1) * D)
                    nc.tensor.matmul(
                        sp[:, h * 512 + coff:(h + 1) * 512],
                        lhsT=kT_sb[rows, b * T + j * 128:b * T + (j + 1) * 128],
                        rhs=qT_sb[rows, b * T + s * 512 + coff:b * T + (s + 1) * 512],
                        start=True, stop=True,
                    )
                sp2 = sp.rearrange("p (h q) -> p h q", h=HPC)
                nc.scalar.activation(
                    out=pt[:, :, j % 2, coff:512], in_=sp2[:, :, coff:512],
                    func=mybir.ActivationFunctionType.Exp, bias=nbias,
                )
                if dj >= 0:
                    # causal zeroing: upper triangle of the diagonal block
                    for h in range(HPC):
                        nc.gpsimd.affine_select(
                            out=pt[:, h, j % 2, coff:coff + 128],
                            in_=pt[:, h, j % 2, coff:coff + 128],
                            compare_op=mybir.AluOpType.is_ge,
                            fill=0.0, base=0,
                            pattern=[[1, 128]], channel_multiplier=-1,
                        )
                # pull PE filler to cover the Act exp drain (~600ns/j)
                want = 600 if j < nj - 2 else 250
                got = 0
                while filler and got < want:
                    ns_est, fn = filler.popleft()
                    fn()
                    got += max(ns_est, 60)

        # ---- PV(idx) + normalization, as filler quanta ----
        def pv_quanta(idx, pts):
            b, s = divmod(idx, NS)
            nj = 4 * s + 4
            quanta = []
            st = {}

            def mk_pv(h, j):
                def fn():
                    if j == 0:
                        st[h] = yp.tile([D + 1, 512], F32, tag="ytl",
                                        name=f"ytl_{idx}_{h}")
                    coff = max(j - 4 * s, 0) * 128
                    nc.tensor.matmul(
                        st[h][:, coff:512],
                        lhsT=vext_sb[:, b * NTB + j, h, :],
                        rhs=pts[j // 2][:, h, j % 2, coff:512],
                        start=(j == 0), stop=(j == nj - 1),
                    )
                return fn

            def mk_norm(h):
                def fn():
                    ytl = st[h]
                    rcp = lpool.tile([1, 512], F32, tag="rcp",
                                     name=f"rcp_{idx}_{h}")
                    nc.vector.reciprocal(rcp, ytl[D:D + 1, :])
                    rbc = lpool.tile([D, 512], F32, tag="rbc",
                                     name=f"rbc_{idx}_{h}")
                    nc.gpsimd.partition_broadcast(rbc, rcp[0:1, :])
                    rows = slice(h * D, (h + 1) * D)
                    nc.vector.tensor_mul(
                        yT_sb[rows, b * T + s * 512:b * T + (s + 1) * 512],
                        ytl[0:D, :], rbc,
                    )
                return fn

            for h in range(HPC):
                for j in range(nj):
                    quanta.append((120, mk_pv(h, j)))
                quanta.append((0, mk_norm(h)))
            return quanta

        # ---- out(idx): output projection + eviction + DMA ----
        def out_quanta(idx):
            quanta = []
            st = {}

            def mk_half(t, e):
                def fn():
                    i = idx * 4 + t
                    if t == 0 and e == 0:
                        st["ob"] = ostage.tile([128, 4, C], BF16, tag="ob",
                                               name=f"ob_{idx}")
                    op = p1.tile([128, 512], F32, tag="p", name=f"op_{i}_{e}")
                    nc.tensor.matmul(
                        op, lhsT=yT_sb[:, i * 128:(i + 1) * 128],
                        rhs=wp_sb[:, e * 512:(e + 1) * 512],
                        start=True, stop=True,
                    )
                    dst = st["ob"][:, t, e * 512:(e + 1) * 512]
                    if (2 * t + e) % 2 == 0:
                        nc.scalar.copy(out=dst, in_=op)
                    else:
                        nc.vector.tensor_copy(out=dst, in_=op)
                return fn

            def dma():
                nc.sync.dma_start(
                    out=out_r[idx], in_=st["ob"])

            for t in range(4):
                for e in range(2):
                    quanta.append((215, mk_half(t, e)))
            quanta.append((0, dma))
            return quanta

        out_r = out.rearrange("(i n p) e -> i p n e", p=128, n=4)

        # ---- pipelined emission ----
        # startup DMA order: wq + the x0 chunks q needs first, then wk/wv
        # and the rest, so proj(0)'s q matmuls start ~1us in
        emit_const_dmas(("q",))
        load_x(0, chunks=((0, 1), (1, 3), (3, 8)))
        emit_const_dmas(("k", "cos", "sin", "v"))
        load_x(1)
        for _, fn in proj_quanta(0):
            fn()
        nc.sync.dma_start(out=wp_sb, in_=wpT)

        pts_store = {}
        for idx in range(NSPAN + 2):
            filler = deque()
            if 1 <= idx <= NSPAN:
                filler.extend(pv_quanta(idx - 1, pts_store.pop(idx - 1)))
            if 2 <= idx <= NSPAN + 1:
                filler.extend(out_quanta(idx - 2))
            if idx + 1 < NSPAN:
                filler.extend(proj_quanta(idx + 1))
            if idx < NSPAN:
                pts = []
                emit_span_S(idx, filler, pts)
                pts_store[idx] = pts
            while filler:
                _, fn = filler.popleft()
                fn()

    nc.compile()
    return nc


_nc_cache = None


def _get_program():
    global _nc_cache
    if _nc_cache is None:
        _nc_cache = _build_program()
    return _nc_cache


def _host_inputs(x, Wq, bq, Wk, bk, Wv, bv, Wp, bp):
    bf = ml_dtypes.bfloat16
    scale = 1.0 / math.sqrt(D)
    x2 = np.ascontiguousarray(np.asarray(x, np.float32).reshape(BT, C).T)  # [C, BT]
    xT_b = x2.astype(bf)
    cos, sin = _rope_cache_host()  # [T, D/2]

    def swizzle_t(m):  # [T, 32] -> [128, NTB*HPC*32] "(n p) d -> p (n h d)"
        t = m.reshape(NTB, 128, 1, 32).transpose(1, 0, 2, 3)
        t = np.broadcast_to(t, (128, NTB, HPC, 32))
        return np.ascontiguousarray(t.reshape(128, NTB * HPC * 32)).astype(bf)

    def swizzle_w(wT):  # [C, DPC] -> [128, 8*DPC] "(k p) d -> p (k d)"
        return np.ascontiguousarray(
            wT.reshape(8, 128, DPC).transpose(1, 0, 2).reshape(128, 8 * DPC)
        ).astype(bf)

    common = {
        "xT": xT_b,
        "cosP": swizzle_t(cos),
        "sinP": swizzle_t(sin),
    }
    in_maps = []
    for m in range(NCORES):
        sl = slice(m * DPC, (m + 1) * DPC)
        in_maps.append({
            **common,
            "wq_p": swizzle_w((np.asarray(Wq, np.float32)[sl] * scale).T),
            "wk_p": swizzle_w(np.asarray(Wk, np.float32)[sl].T),
            "wv_p": swizzle_w(np.asarray(Wv, np.float32)[sl].T),
            "wpT": np.ascontiguousarray(
                np.asarray(Wp, np.float32)[:, sl].T).astype(bf),
        })
    return in_maps


def kernel(x, Wq, bq, Wk, bk, Wv, bv, Wp, bp, _run_kwargs=None):
    nc = _get_program()
    in_maps = _host_inputs(x, Wq, bq, Wk, bk, Wv, bv, Wp, bp)
    res = run_bass_kernel_spmd(
        nc, in_maps, core_ids=list(range(NCORES)), **(_run_kwargs or {})
    )
    partials = [r["out_p"] for r in res.results]
    acc = np.zeros((BT, C), np.float32)
    for p in partials:
        acc += np.asarray(p, dtype=np.float32)
    out = acc + np.asarray(bp, np.float32)[None, :]
    if _run_kwargs:
        kernel.last_results = res
    return out.reshape(B, T, C)
